# revision 14
# baseline (speedup 1.0000x reference)
"""Trainium2 Bass kernel for nn_Attention_40020505264416.

Reference computation (B=4, H=16, N=1024, C=64, D=H*C=1024):
    scores = einsum('bhnc,bhmc->bhnm', q, k) * C**-0.5
    attn   = pe + softmax(scores, axis=-1)          # post-softmax bias
    ctx    = einsum('bhnm,bhmc->bhnc', attn, v)
    x      = ctx.transpose(0,2,1,3).reshape(B, N, D)
    out    = silu(x @ w1 + b1) @ w2 + b2

Distribution: pure data-parallel over query rows (N sharded 8-way, 128
rows per core).  Each core receives full K/V (pre-transposed on host),
its slice of q/pe, and full MLP weights; there is no inter-core
communication.  All device-side layouts are produced on the host so the
device never transposes a large tensor:

  qT  [B,H,C,NS]   q^T slices         (lhs of S^T = k @ q^T contraction)
  kT  [B,H,C,N]    k^T                (stationary operand of QK)
  vp  [H,N,B,C+1]  v with a ones column appended -> AV matmul emits the
                   softmax denominator as psum column 64 for free
  peT [H,N,NS]     pe^T slices        (stationary operand of pe @ v)

Per (b,h) pair on device:
  S^T[m,q]  : 8 matmuls  lhsT=kT chunk [64,128],  rhs=qT [64,128].
              The two batches of a qk tile sit at SBUF partitions 0:64
              and 64:128, so their K=64 matmuls carry tile_position
              (0,0) / (64,0); interleaving the j loop over both batches
              makes adjacent matmuls target disjoint PE row groups and
              run CONCURRENTLY (the array is 16 32x32 subarrays) --
              QK cost per pair drops ~2x vs. the serial order.
  expS      : one ACT Exp over [128, 8*128] psum -> sbuf (scale=C**-0.5)
  ctx_exp   : 8 matmuls  lhsT=expS chunk,         rhs=vp[:,j,b,:] ([128,65])
              -> psum [q, 65]; col 64 = softmax denominator
  ctx_pe    : 8 matmuls  lhsT=peT chunk,          rhs=vp[:,j,:,:] ([128,4*65])
              (batched over b; shared across the 4 batches of the head)
  x[q, h*C:..] = ctx_exp[:, :64] * (1/den) + ctx_pe[:, b, :64]   (one DVE op)

PSUM budget (8 banks): S^T ring 3 x 2 banks; av/pe4/pt share one tag,
2 x 1 bank.

MLP (rows = (b, q) = 512 per core):
  xT chunks via 32 PE transposes, fc1 emits hdn^T directly
  (lhsT = w1 chunk, rhs = xT chunk), Silu+b1 fused in ONE ACT eviction,
  fc2 consumes hdn^T chunks as lhsT and writes natural [rows, d] psum
  tiles that DMA straight to DRAM.  b2 is added during the DVE psum
  eviction from a broadcast tile built with two K=1 ones matmuls, and
  the output DMAs alternate between the scalar and sync queues so the
  final transfers drain in parallel.

Perf notes (measured on HW, 8-core SPMD):
  - v1 (serial QK, bufs 2/2/2): 154921-158063ns.  PE busy 131.8us of
    161us span; QK = 512 K=64 matmuls at ~70ns each.
  - ALL input DMAs ride ONE queue (sync) in exact consumption order:
    ident, then per head [qk(b01), pv, qk(b23)].  With qk/pv/ident on
    three separate queues they raced each other for HBM; a late
    transfer stalled the PE >3.4us and tripped the HAM activity
    monitor into 1.2GHz half-clock windows costing 12-80us, with huge
    run-to-run variance (149-185us).  Exception: the first three
    heads' pv transfers go on the otherwise-idle gpsimd queue IN
    PARALLEL, because early per-queue DMA bandwidth (~100GB/s) makes
    the serialized 794KB pv transfer arrive ~5us too late for pe4.
  - MLP weight chunks stream on the scalar queue during heads 2..9 so
    they can never head-of-line-block the latency-critical kT/qT DMAs.
  - Things that were tried and REGRESSED (do not re-attempt blindly):
    splitting the exp into half-tiles (PSUM bank cycling trips the PE
    HAM activity monitor into long 1.2GHz windows), 64x128 row-tiled
    batch-interleaved S^T (no speedup - stream-bound, not array-bound),
    pv prefetch depth < 12 (mid-kernel starvation), per-pair filler
    matmuls, and moving the MLP weight stream to heads 6..13.
"""

import os
import sys

for _p in ("/opt/trn_rl_repo",):
    if os.path.isdir(_p) and _p not in sys.path:
        sys.path.insert(0, _p)

import numpy as np

import concourse.bass as bass
import concourse.mybir as mybir
import concourse.tile as tile
from concourse import bacc
from concourse.bass_utils import run_bass_kernel_spmd

B, H, N, C = 4, 16, 1024, 64
D = H * C
NCORES = 8
NS = N // NCORES          # query rows per core
J = N // 128              # key chunks of 128
SCALE = C ** -0.5
# DVE fast-exp (Schraudolph in bf16 bit space): bf16(int16(A*x + B)) ~= e^x
# with ~4% sawtooth error; softmax self-normalizes, end-to-end ~1e-4.
FE_A = SCALE * 128.0 / np.log(2.0)
FE_B = 127.0 * 128.0 - 4.75

PVW = NS + B * (C + 1)       # packed peT|v' row width
F32 = mybir.dt.float32
# Compute dtype for matmul operands (host pre-casts inputs to this).
CDT = mybir.dt.bfloat16 if os.environ.get("KERNEL_DT", "bf16") == "bf16" else F32
# q/k ride fp8-e4m3: the softmax path contributes ~0.2% of ctx magnitude
# (pe@v dominates), so fp8 scores cost 9e-5 rel err and halve the
# dominant DMA stream.
QDT = mybir.dt.float8e4


def build_program(cdt=CDT, qdt=QDT):
    nc = bacc.Bacc(None, debug=False)

    # k^T and q^T packed in one tensor, two batches stacked on the
    # partition axis: [h, b//2, (b%2)*C+c, 0:N]=kT, [.., N:N+NS]=qT
    qk_d = nc.dram_tensor("qk", [H, B // 2, 2 * C, N + NS], qdt,
                          kind="ExternalInput")
    # pe^T and v' packed per head, HOST-swizzled so the device AP is
    # contiguous per partition: [h, p, j, x] with n = j*128+p.  (The
    # device-side rearrange gather cost 0.9-3.4us of DIRECT2D descriptor
    # generation PER TRANSFER on the issuing sequencer, head-of-line
    # blocking the latency-critical qk triggers.)
    pv_d = nc.dram_tensor("pv", [H, 128, J, PVW], cdt, kind="ExternalInput")
    idm_d = nc.dram_tensor("idm", [128, 128], cdt, kind="ExternalInput")
    # w1/w2/b1 likewise host-swizzled to [p, ...] partition-major.
    w1_d = nc.dram_tensor("w1s", [128, D // 128, D], cdt,
                          kind="ExternalInput")
    b1_d = nc.dram_tensor("b1s", [128, D // 128], F32, kind="ExternalInput")
    w2_d = nc.dram_tensor("w2s", [128, D // 128, D], cdt,
                          kind="ExternalInput")
    b2_d = nc.dram_tensor("b2s", [1, D], cdt, kind="ExternalInput")
    out_d = nc.dram_tensor("out", [B, NS, D], F32, kind="ExternalOutput")

    with tile.TileContext(nc) as tc:
        from contextlib import ExitStack

        with ExitStack() as ctx:
            const = ctx.enter_context(tc.tile_pool(name="const", bufs=1))

            # ident rides FIRST on the sync queue (32KB, negligible): on the
            # scalar queue its transfer starved behind the early qk/pv flood
            # and the first x^T transpose waited ~8us for it.
            ident = const.tile([128, 128], cdt, tag="ident")
            nc.sync.dma_start(ident[:], idm_d[:])
            ones1 = const.tile([1, 128], cdt, tag="ones1")
            nc.vector.memset(ones1[:], 1.0)

            # MLP weights: DMA'd in D//128 chunks interleaved into the
            # attention h-loop (sync/HWDGE queue) so the 4MB doesn't
            # head-of-line-block the per-pair kT/qT stream.
            w1_s = const.tile([128, D // 128, D], cdt, tag="w1s")
            w2_s = const.tile([128, D // 128, D], cdt, tag="w2s")
            w1_r = w1_d
            w2_r = w2_d
            b1_s = const.tile([128, D // 128], F32, tag="b1s")
            nc.scalar.dma_start(b1_s[:], b1_d[:])
            b2_s = const.tile([1, D], cdt, tag="b2s")
            nc.scalar.dma_start(b2_s[:], b2_d[:])

            # HAM warm-up fodder: keeps the PE activity window full while
            # the first attention DMAs land, so the clock ramps to 8/8
            # early instead of at the MLP phase.
            warm_w = const.tile([128, 128], cdt, tag="warmw", name="warm_w")
            nc.vector.memset(warm_w[:], 0.0)
            warm_r = const.tile([128, 512], cdt, tag="warmr", name="warm_r")
            nc.vector.memset(warm_r[:], 0.0)

            # b2 broadcast to all 128 partitions (fc2 adds it during the
            # DVE psum eviction instead of a K=1 matmul per output tile).
            b2b = const.tile([128, D], F32, tag="b2b", name="b2b")

            # Attention output, natural layout [q, d] per batch.
            x_nat = [const.tile([NS, H, C], cdt, tag=f"xnat{b}", name=f"xnat{b}")
                     for b in range(B)]
            # x^T chunks [d-in-chunk, chunk, b, q] and hdn^T chunks.
            xT = const.tile([128, D // 128, B, NS], cdt, tag="xT")
            hdnT = const.tile([128, D // 128, B, NS], cdt, tag="hdnT")

            # ---------------- attention ----------------
            with ExitStack() as attn_ctx:
                pool_pe = attn_ctx.enter_context(tc.tile_pool(name="pe", bufs=4))
                pool_v = attn_ctx.enter_context(tc.tile_pool(name="v", bufs=14))
                pool_k = attn_ctx.enter_context(tc.tile_pool(name="k", bufs=8))
                pool_e = attn_ctx.enter_context(tc.tile_pool(name="e", bufs=5))
                pool_r = attn_ctx.enter_context(tc.tile_pool(name="r", bufs=4))
                # S^T ring: 3 tiles of 2 banks.  av / pe4 / pt share one
                # 1-bank tag with 2 bufs -> exactly 8 PSUM banks total.
                psum_s = attn_ctx.enter_context(
                    tc.tile_pool(name="ps", bufs=3, space="PSUM"))
                psum_m = attn_ctx.enter_context(
                    tc.tile_pool(name="pm", bufs=2, space="PSUM"))

                # Ramp prefetch: head 2's whole supply (qk x2 + pv) rides
                # the scalar queue, which is otherwise idle until the MLP
                # weight stream (~30us).  Early per-queue DMA bandwidth is
                # only ~100GB/s, so sync (ident + heads 0-1 qk) and gpsimd
                # (heads 0-1 pv) alone cannot feed three heads in time.
                # pv(h0), pv(h1) FIRST on gpsimd: pe4(h0) is the
                # earliest pv consumer; everything else queues behind.
                pre_pv01 = []
                for ph in range(2):
                    t = pool_v.tile([128, J, PVW], cdt, tag="vp",
                                    name=f"prepv{ph}")
                    nc.gpsimd.dma_start(t[:], pv_d[ph])
                    pre_pv01.append(t)
                pre_qk = {}
                for (ph, pbp) in ((1, 1), (2, 0), (2, 1)):
                    t = pool_k.tile([2 * C, N + NS], qdt, tag="kT",
                                    name=f"preqk{ph}{pbp}")
                    nc.gpsimd.dma_start(t[:], qk_d[ph, pbp])
                    pre_qk[(ph, pbp)] = t
                pre_pv = pool_v.tile([128, J, PVW], cdt, tag="vp",
                                     name="pre_pv")
                nc.gpsimd.dma_start(pre_pv[:], pv_d[2])

                # Small ramp block: first real QK lands ~10us in; a short
                # burst warms the HAM clock without delaying it (the old
                # 16-MM block pushed the first QK to 14us+).
                for w in range(4):
                    wt = psum_s.tile([128, 512], F32, tag="st", name="warm_t")
                    nc.tensor.matmul(wt[:], warm_w[:], warm_r[:],
                                     start=True, stop=True)

                def do_av(pairs):
                    """AV matmuls + normalization fixups for up to two
                    finished pairs, BATCHED into one psum tile with one
                    shared reciprocal: the av/pe4/pt psum tag rotates only
                    2 banks, and every allocation costs a PE<->DVE
                    semaphore round trip -- fewer, fatter allocations keep
                    that ring off the critical path.

                    Emitted one group late so the PE never waits on the
                    exp of the current group (software pipelining)."""
                    n = len(pairs)
                    av = psum_m.tile([NS, 2, C + 1], F32, tag="m", name="av")
                    for i, (h, b, expS, vp_p, pe4_sb_p) in enumerate(pairs):
                        for j in range(J):
                            nc.tensor.matmul(
                                av[:, i, :], expS[:, j, :], vp_p[:, j, b, :],
                                start=(j == 0), stop=(j == J - 1))
                    recip = pool_r.tile([NS, 2, 1], F32, tag="recip",
                                        name="recip")
                    nc.vector.reciprocal(recip[:, 0:n, :],
                                         av[:, 0:n, C:C + 1])
                    for i, (h, b, expS, vp_p, pe4_sb_p) in enumerate(pairs):
                        # x = ctx_exp/den + ctx_pe
                        nc.vector.scalar_tensor_tensor(
                            out=x_nat[b][:, h, :],
                            in0=av[:, i, 0:C],
                            scalar=recip[:, i, 0:1],
                            in1=pe4_sb_p[:, b, 0:C],
                            op0=mybir.AluOpType.mult,
                            op1=mybir.AluOpType.add)


                def qk_mms(items):
                    """Emit j-interleaved QK matmuls for the given
                    (st, qk_t, lo) operands: adjacent matmuls hit
                    disjoint PE row groups (rows 0:64 / 64:128) and run
                    concurrently on the 16-subarray PE."""
                    for j in range(J):
                        for st_t, qk_s, lo in items:
                            nc.tensor.matmul(
                                st_t[:, j, :],
                                qk_s[lo:lo + C, j * 128:(j + 1) * 128],
                                qk_s[lo:lo + C, N:],
                                start=True, stop=True,
                                tile_position=(lo, 0))

                def do_exp(st_t, h, b, vp_p, pe4_p):
                    e = pool_e.tile([128, J, NS], cdt, tag="expS", name="e")
                    nc.scalar.activation(
                        e[:], st_t[:],
                        mybir.ActivationFunctionType.Exp, scale=SCALE)
                    return (h, b, e, vp_p, pe4_p)

                # Software-pipelined B tile: iteration g emits matmuls for
                # (stB of group g-1, stA of group g).  Both tiles' ring
                # buffers were freed >=3 exp-slots earlier, so the QK
                # stream NEVER waits on the exp pacer -- exp(B_{g-1}) can
                # start the instant exp(A_{g-1}) finishes.
                bq = None           # pending B work: (stB, qk_t, h, b, vp, pe4)
                exp_prev = []       # pairs exp'd last iteration -> AV now
                exp_cur = []
                for h in range(H):
                    pe4_sb = pool_pe.tile([NS, B, C + 1], F32, tag="pe4sb",
                                          name="pe4_sb")

                    for bp in range(2):
                        # k^T|q^T for TWO batches stacked on the partition
                        # axis: one full-128-partition DMA per two pairs.
                        if (h, bp) in pre_qk:
                            qk_t = pre_qk[(h, bp)]
                        else:
                            qk_t = pool_k.tile([2 * C, N + NS], qdt,
                                               tag="kT")
                            nc.sync.dma_start(qk_t[:], qk_d[h, bp])
                        if bp == 0:
                            if h == 2:
                                pv_t = pre_pv
                            elif h < 2:
                                pv_t = pre_pv01[h]
                            else:
                                pv_t = pool_v.tile([128, J, PVW], cdt,
                                                   tag="vp", name="pv_t")
                                nc.gpsimd.dma_start(pv_t[:], pv_d[h])
                            peT_t = pv_t[:, :, 0:NS]
                            vp_t = pv_t[:, :, NS:].rearrange(
                                "p j (b c) -> p j b c", b=B)

                        stA = psum_s.tile([128, J, NS], F32, tag="st",
                                          name="stA")
                        stB = psum_s.tile([128, J, NS], F32, tag="st",
                                          name="stB")
                        mms = [(stA, qk_t, 0)]
                        if bq is not None:
                            mms.insert(0, (bq[0], bq[1], C))
                        qk_mms(mms)

                        exp_cur = []
                        if bq is not None:
                            exp_cur.append(do_exp(bq[0], bq[2], bq[3],
                                                  bq[4], bq[5]))
                        exp_cur.append(do_exp(stA, h, 2 * bp, vp_t, pe4_sb))

                        if exp_prev:
                            do_av(exp_prev)
                        exp_prev = exp_cur
                        if h < 4:
                            # ramp-phase filler: keep the PE activity window
                            # full while the pipeline is still shallow
                            for _ in range(8 if h < 2 else 2):
                                wt = psum_s.tile([128, 512], F32, tag="st",
                                                 name="warm_t")
                                nc.tensor.matmul(wt[:], warm_w[:], warm_r[:],
                                                 start=True, stop=True)
                        bq = (stB, qk_t, h, 2 * bp + 1, vp_t, pe4_sb)

                        if bp == 0:
                            # pe @ v for all 4 batches of this head,
                            # emitted after ready PE work so a late vp/peT
                            # DMA can't stall the in-order PE stream.
                            pe4 = psum_m.tile([NS, B, C + 1], F32,
                                              tag="m", name="pe4")
                            for j in range(J):
                                nc.tensor.matmul(
                                    pe4[:], peT_t[:, j, :], vp_t[:, j, :, :],
                                    start=(j == 0), stop=(j == J - 1))
                            # stage in SBUF: DVE may read only one PSUM
                            # input (DMA and GPSIMD cannot read PSUM at all)
                            nc.vector.tensor_copy(pe4_sb[:], pe4[:])
                        elif 2 <= h < 10:
                            # stream two MLP weight chunks per head on the
                            # gpsimd queue -- never on the ACT/sync
                            # sequencers, whose pacing is latency-critical.
                            for wc in range(2):
                                ci = (h - 2) * 2 + wc
                                if ci < D // 128:
                                    nc.sync.dma_start(w1_s[:, ci, :],
                                                      w1_r[:, ci, :])
                                else:
                                    nc.sync.dma_start(
                                        w2_s[:, ci - D // 128, :],
                                        w2_r[:, ci - D // 128, :])
                # drain the pipeline: B of the last group, then final AVs
                qk_mms([(bq[0], bq[1], C)])
                last = do_exp(bq[0], bq[2], bq[3], bq[4], bq[5])
                do_av(exp_prev)
                do_av([last])

            # ---------------- MLP ----------------
            with ExitStack() as mlp_ctx:
                psum_h1 = mlp_ctx.enter_context(
                    tc.tile_pool(name="ph1", bufs=3, space="PSUM"))
                psum_y = mlp_ctx.enter_context(
                    tc.tile_pool(name="py", bufs=4, space="PSUM"))

                # x^T via 32 PE transposes, moved OUT of the attention
                # steady state: the av/pe4 psum tag there rotates only 2
                # banks and every pt allocation cost a PE<->DVE semaphore
                # round trip that stalled the exp pacer.
                for t in range(D // 128):
                    for b in range(B):
                        pt = psum_y.tile([128, NS], cdt, tag="y", name="pt")
                        nc.tensor.transpose(
                            pt[:], x_nat[b][:, 2 * t:2 * t + 2, :], ident[:])
                        nc.vector.tensor_copy(xT[:, t, b, :], pt[:])

                # b2 broadcast tile: [128, D] = ones_col @ b2 (two K=1
                # matmuls); fc2 adds it during the DVE psum eviction.
                for half in range(2):
                    bb = psum_y.tile([128, 512], F32, tag="y", name="bb")
                    nc.tensor.matmul(
                        bb[:], ones1[:1, :],
                        b2_s[:1, half * 512:(half + 1) * 512],
                        start=True, stop=True)
                    nc.vector.tensor_copy(
                        b2b[:, half * 512:(half + 1) * 512], bb[:])

                # fc1: hdn^T[do, rows] = sum_i w1[i]^T.T @ xT[i]
                # Silu(h1 + b1) in ONE fused ACT eviction (bias is
                # per-partition in this layout) -- the old sigmoid+DVE
                # chain made fc1 evacuation-bound.
                for o in range(D // 128):
                    h1 = psum_h1.tile([128, B, NS], F32, tag="h1")
                    for i in range(D // 128):
                        nc.tensor.matmul(
                            h1[:], w1_s[:, i, o * 128:(o + 1) * 128],
                            xT[:, i, :, :],
                            start=(i == 0), stop=(i == D // 128 - 1))
                    nc.scalar.activation(
                        hdnT[:, o, :, :], h1[:],
                        mybir.ActivationFunctionType.Silu,
                        bias=b1_s[:, o:o + 1])

                # fc2: y[rows, do] = sum_i hdnT[i].T @ w2[i]; b2 is added
                # by the DVE during psum eviction (b2b broadcast tile).
                pool_o = mlp_ctx.enter_context(tc.tile_pool(name="o", bufs=4))
                for t in range(B):
                    for nn in range(2):
                        y = psum_y.tile([128, 512], F32, tag="y")
                        for i in range(D // 128):
                            nc.tensor.matmul(
                                y[:], hdnT[:, i, t, :],
                                w2_s[:, i, nn * 512:(nn + 1) * 512],
                                start=(i == 0), stop=(i == D // 128 - 1))
                        y_sb = pool_o.tile([128, 512], F32, tag="ysb")
                        nc.vector.tensor_tensor(
                            out=y_sb[:], in0=y[:],
                            in1=b2b[:, nn * 512:(nn + 1) * 512],
                            op=mybir.AluOpType.add)
                        # alternate output DMAs across two queues so the
                        # final transfers drain in parallel
                        eng = (nc.scalar, nc.sync,
                               nc.gpsimd)[(t * 2 + nn) % 3]
                        eng.dma_start(
                            out_d[t, :, nn * 512:(nn + 1) * 512], y_sb[:])

    nc.compile()
    return nc


_PROG = None


def _get_prog():
    global _PROG
    if _PROG is None:
        _PROG = build_program()
    return _PROG


def _np_dt(cdt):
    if cdt == mybir.dt.bfloat16:
        import ml_dtypes
        return ml_dtypes.bfloat16
    return np.float32


def make_in_maps(q, k, v, pe, w1, b1, w2, b2, cdt=CDT):
    import ml_dtypes
    ndt = _np_dt(cdt)
    qdt8 = ml_dtypes.float8_e4m3
    # [b,h,n,c] -> [h, b//2, (b%2)*C+c, n]
    qT = np.transpose(q, (1, 0, 3, 2)).reshape(H, B // 2, 2 * C, N)
    qT = qT.astype(qdt8)
    kT = np.transpose(k, (1, 0, 3, 2)).reshape(H, B // 2, 2 * C, N)
    kT = kT.astype(qdt8)
    vp = np.concatenate([v, np.ones((B, H, N, 1), v.dtype)], axis=-1)
    vp = np.transpose(vp, (1, 2, 0, 3)).reshape(H, N, B * (C + 1)).astype(ndt)
    peT = np.transpose(pe[0], (0, 2, 1)).astype(ndt)
    # w1/w2 host-swizzled to [p, i, o] with din = i*128+p; b1 to [p, o128]
    w1c = np.ascontiguousarray(
        w1.reshape(D // 128, 128, D).transpose(1, 0, 2)).astype(ndt)
    w2c = np.ascontiguousarray(
        w2.reshape(D // 128, 128, D).transpose(1, 0, 2)).astype(ndt)
    b1f = np.ascontiguousarray(
        b1.reshape(D // 128, 128).T).astype(np.float32)
    b2c = np.ascontiguousarray(b2.reshape(1, D)).astype(ndt)
    idm = np.eye(128, dtype=np.float32).astype(ndt)

    in_maps = []
    for r in range(NCORES):
        sl = slice(r * NS, (r + 1) * NS)
        # kT is full N (not sharded); qT carries this core's q rows
        qk = np.ascontiguousarray(
            np.concatenate([kT, qT[:, :, :, sl]], axis=-1))
        pv = np.concatenate([peT[:, :, sl], vp], axis=-1)
        # swizzle n -> (j, p): [H, N, PVW] -> [H, 128, J, PVW] so the
        # device DMA is contiguous per partition (no DIRECT2D gather).
        pv = np.ascontiguousarray(
            pv.reshape(H, J, 128, PVW).transpose(0, 2, 1, 3))
        in_maps.append({
            "qk": qk,
            "pv": pv,
            "idm": idm,
            "w1s": w1c,
            "b1s": b1f,
            "w2s": w2c,
            "b2s": b2c,
        })
    return in_maps


def assemble(results):
    out = np.empty((B, N, D), np.float32)
    for r in range(NCORES):
        out[:, r * NS:(r + 1) * NS, :] = results[r]["out"]
    return out


def kernel(q, k, v, pe, w1, b1, w2, b2):
    nc = _get_prog()
    in_maps = make_in_maps(q, k, v, pe, w1, b1, w2, b2)
    res = run_bass_kernel_spmd(nc, in_maps, core_ids=list(range(NCORES)))
    return assemble(res.results)


# revision 15
# speedup vs baseline: 1.2580x; 1.2580x over previous
"""Trainium2 Bass kernel for nn_Attention_40020505264416.

Reference computation (B=4, H=16, N=1024, C=64, D=H*C=1024):
    scores = einsum('bhnc,bhmc->bhnm', q, k) * C**-0.5
    attn   = pe + softmax(scores, axis=-1)          # post-softmax bias
    ctx    = einsum('bhnm,bhmc->bhnc', attn, v)
    x      = ctx.transpose(0,2,1,3).reshape(B, N, D)
    out    = silu(x @ w1 + b1) @ w2 + b2

Distribution: pure data-parallel over query rows (N sharded 8-way, 128
rows per core).  Each core receives full K/V (pre-transposed on host),
its slice of q/pe, and full MLP weights; there is no inter-core
communication.  All device-side layouts are produced on the host so the
device never transposes a large tensor:

  qT  [B,H,C,NS]   q^T slices         (lhs of S^T = k @ q^T contraction)
  kT  [B,H,C,N]    k^T                (stationary operand of QK)
  vp  [H,N,B,C+1]  v with a ones column appended -> AV matmul emits the
                   softmax denominator as psum column 64 for free
  peT [H,N,NS]     pe^T slices        (stationary operand of pe @ v)

Per (b,h) pair on device:
  S^T[m,q]  : 8 matmuls  lhsT=kT chunk [64,128],  rhs=qT [64,128].
              The two batches of a qk tile sit at SBUF partitions 0:64
              and 64:128, so their K=64 matmuls carry tile_position
              (0,0) / (64,0); interleaving the j loop over both batches
              makes adjacent matmuls target disjoint PE row groups and
              run CONCURRENTLY (the array is 16 32x32 subarrays) --
              QK cost per pair drops ~2x vs. the serial order.
  expS      : one ACT Exp over [128, 8*128] psum -> sbuf (scale=C**-0.5)
  ctx_exp   : 8 matmuls  lhsT=expS chunk,         rhs=vp[:,j,b,:] ([128,65])
              -> psum [q, 65]; col 64 = softmax denominator
  ctx_pe    : 8 matmuls  lhsT=peT chunk,          rhs=vp[:,j,:,:] ([128,4*65])
              (batched over b; shared across the 4 batches of the head)
  x[q, h*C:..] = ctx_exp[:, :64] * (1/den) + ctx_pe[:, b, :64]   (one DVE op)

PSUM budget (8 banks): S^T ring 3 x 2 banks; av/pe4/pt share one tag,
2 x 1 bank.

MLP (rows = (b, q) = 512 per core):
  xT chunks via 32 PE transposes, fc1 emits hdn^T directly
  (lhsT = w1 chunk, rhs = xT chunk), Silu+b1 fused in ONE ACT eviction,
  fc2 consumes hdn^T chunks as lhsT and writes natural [rows, d] psum
  tiles that DMA straight to DRAM.  b2 is added during the DVE psum
  eviction from a broadcast tile built with two K=1 ones matmuls, and
  the output DMAs alternate between the scalar and sync queues so the
  final transfers drain in parallel.

Perf notes (measured on HW, 8-core SPMD):
  - v1 (serial QK, bufs 2/2/2): 154921-158063ns.  PE busy 131.8us of
    161us span; QK = 512 K=64 matmuls at ~70ns each.
  - ALL input DMAs ride ONE queue (sync) in exact consumption order:
    ident, then per head [qk(b01), pv, qk(b23)].  With qk/pv/ident on
    three separate queues they raced each other for HBM; a late
    transfer stalled the PE >3.4us and tripped the HAM activity
    monitor into 1.2GHz half-clock windows costing 12-80us, with huge
    run-to-run variance (149-185us).  Exception: the first three
    heads' pv transfers go on the otherwise-idle gpsimd queue IN
    PARALLEL, because early per-queue DMA bandwidth (~100GB/s) makes
    the serialized 794KB pv transfer arrive ~5us too late for pe4.
  - MLP weight chunks stream on the scalar queue during heads 2..9 so
    they can never head-of-line-block the latency-critical kT/qT DMAs.
  - Things that were tried and REGRESSED (do not re-attempt blindly):
    splitting the exp into half-tiles (PSUM bank cycling trips the PE
    HAM activity monitor into long 1.2GHz windows), 64x128 row-tiled
    batch-interleaved S^T (no speedup - stream-bound, not array-bound),
    pv prefetch depth < 12 (mid-kernel starvation), per-pair filler
    matmuls, and moving the MLP weight stream to heads 6..13.
"""

import os
import sys

for _p in ("/opt/trn_rl_repo",):
    if os.path.isdir(_p) and _p not in sys.path:
        sys.path.insert(0, _p)

import numpy as np

import concourse.bass as bass
import concourse.mybir as mybir
import concourse.tile as tile
from concourse import bacc
from concourse.bass_utils import run_bass_kernel_spmd

B, H, N, C = 4, 16, 1024, 64
D = H * C
NCORES = 8
NS = N // NCORES          # query rows per core
J = N // 128              # key chunks of 128
SCALE = C ** -0.5
# DVE fast-exp (Schraudolph in bf16 bit space): bf16(int16(A*x + B)) ~= e^x
# with ~4% sawtooth error; softmax self-normalizes, end-to-end ~1e-4.
FE_A = SCALE * 128.0 / np.log(2.0)
FE_B = 127.0 * 128.0 - 4.75

PVW = NS + B * (C + 1)       # packed peT|v' row width
F32 = mybir.dt.float32
# Compute dtype for matmul operands (host pre-casts inputs to this).
CDT = mybir.dt.bfloat16 if os.environ.get("KERNEL_DT", "bf16") == "bf16" else F32
# q/k ride fp8-e4m3: the softmax path contributes ~0.2% of ctx magnitude
# (pe@v dominates), so fp8 scores cost 9e-5 rel err and halve the
# dominant DMA stream.
QDT = mybir.dt.float8e4


def build_program(cdt=CDT, qdt=QDT):
    nc = bacc.Bacc(None, debug=False)

    # k^T and q^T packed in one tensor, two batches stacked on the
    # partition axis: [h, b//2, (b%2)*C+c, 0:N]=kT, [.., N:N+NS]=qT
    qk_d = nc.dram_tensor("qk", [H, B // 2, 2 * C, N + NS], qdt,
                          kind="ExternalInput")
    # pe^T and v' packed per head, HOST-swizzled so the device AP is
    # contiguous per partition: [h, p, j, x] with n = j*128+p.  (The
    # device-side rearrange gather cost 0.9-3.4us of DIRECT2D descriptor
    # generation PER TRANSFER on the issuing sequencer, head-of-line
    # blocking the latency-critical qk triggers.)
    pv_d = nc.dram_tensor("pv", [H, 128, J, PVW], cdt, kind="ExternalInput")
    idm_d = nc.dram_tensor("idm", [128, 128], cdt, kind="ExternalInput")
    # w1/w2/b1 likewise host-swizzled to [p, ...] partition-major.
    w1_d = nc.dram_tensor("w1s", [128, D // 128, D], cdt,
                          kind="ExternalInput")
    b1_d = nc.dram_tensor("b1s", [128, D // 128], F32, kind="ExternalInput")
    w2_d = nc.dram_tensor("w2s", [128, D // 128, D], cdt,
                          kind="ExternalInput")
    b2_d = nc.dram_tensor("b2s", [1, D], cdt, kind="ExternalInput")
    out_d = nc.dram_tensor("out", [B, NS, D], F32, kind="ExternalOutput")

    with tile.TileContext(nc) as tc:
        from contextlib import ExitStack

        with ExitStack() as ctx:
            const = ctx.enter_context(tc.tile_pool(name="const", bufs=1))

            # ident rides FIRST on the sync queue (32KB, negligible): on the
            # scalar queue its transfer starved behind the early qk/pv flood
            # and the first x^T transpose waited ~8us for it.
            ident = const.tile([128, 128], cdt, tag="ident")
            nc.sync.dma_start(ident[:], idm_d[:])
            ones1 = const.tile([1, 128], cdt, tag="ones1")
            nc.vector.memset(ones1[:], 1.0)

            # MLP weights: DMA'd in D//128 chunks interleaved into the
            # attention h-loop (sync/HWDGE queue) so the 4MB doesn't
            # head-of-line-block the per-pair kT/qT stream.
            w1_s = const.tile([128, D // 128, D], cdt, tag="w1s")
            w2_s = const.tile([128, D // 128, D], cdt, tag="w2s")
            w1_r = w1_d
            w2_r = w2_d
            b1_s = const.tile([128, D // 128], F32, tag="b1s")
            nc.scalar.dma_start(b1_s[:], b1_d[:])
            b2_s = const.tile([1, D], cdt, tag="b2s")
            nc.scalar.dma_start(b2_s[:], b2_d[:])

            # HAM warm-up fodder: keeps the PE activity window full while
            # the first attention DMAs land, so the clock ramps to 8/8
            # early instead of at the MLP phase.
            warm_w = const.tile([128, 128], cdt, tag="warmw", name="warm_w")
            nc.vector.memset(warm_w[:], 0.0)
            warm_r = const.tile([128, 512], cdt, tag="warmr", name="warm_r")
            nc.vector.memset(warm_r[:], 0.0)

            # b2 broadcast to all 128 partitions (fc2 adds it during the
            # DVE psum eviction instead of a K=1 matmul per output tile).
            b2b = const.tile([128, D], F32, tag="b2b", name="b2b")

            # Attention output, natural layout [q, d] per batch.
            x_nat = [const.tile([NS, H, C], cdt, tag=f"xnat{b}", name=f"xnat{b}")
                     for b in range(B)]
            # x^T chunks [d-in-chunk, chunk, b, q] and hdn^T chunks.
            xT = const.tile([128, D // 128, B, NS], cdt, tag="xT")
            hdnT = const.tile([128, D // 128, B, NS], cdt, tag="hdnT")

            # ---------------- attention ----------------
            with ExitStack() as attn_ctx:
                pool_pe = attn_ctx.enter_context(tc.tile_pool(name="pe", bufs=4))
                pool_v = attn_ctx.enter_context(tc.tile_pool(name="v", bufs=12))
                pool_k = attn_ctx.enter_context(tc.tile_pool(name="k", bufs=8))
                pool_e = attn_ctx.enter_context(tc.tile_pool(name="e", bufs=5))
                pool_r = attn_ctx.enter_context(tc.tile_pool(name="r", bufs=4))
                # S^T ring: 3 tiles of 2 banks.  av / pe4 / pt share one
                # 1-bank tag with 2 bufs -> exactly 8 PSUM banks total.
                psum_s = attn_ctx.enter_context(
                    tc.tile_pool(name="ps", bufs=3, space="PSUM"))
                psum_m = attn_ctx.enter_context(
                    tc.tile_pool(name="pm", bufs=2, space="PSUM"))

                # Ramp prefetch: head 2's whole supply (qk x2 + pv) rides
                # the scalar queue, which is otherwise idle until the MLP
                # weight stream (~30us).  Early per-queue DMA bandwidth is
                # only ~100GB/s, so sync (ident + heads 0-1 qk) and gpsimd
                # (heads 0-1 pv) alone cannot feed three heads in time.
                # pv(h0), pv(h1) FIRST on gpsimd: pe4(h0) is the
                # earliest pv consumer; everything else queues behind.
                pre_pv01 = []
                for ph in range(2):
                    t = pool_v.tile([128, J, PVW], cdt, tag="vp",
                                    name=f"prepv{ph}")
                    nc.gpsimd.dma_start(t[:], pv_d[ph])
                    pre_pv01.append(t)
                pre_qk = {}
                for (ph, pbp) in ((1, 1), (2, 0), (2, 1)):
                    t = pool_k.tile([2 * C, N + NS], qdt, tag="kT",
                                    name=f"preqk{ph}{pbp}")
                    nc.gpsimd.dma_start(t[:], qk_d[ph, pbp])
                    pre_qk[(ph, pbp)] = t
                pre_pv = pool_v.tile([128, J, PVW], cdt, tag="vp",
                                     name="pre_pv")
                nc.gpsimd.dma_start(pre_pv[:], pv_d[2])

                # Small ramp block: first real QK lands ~10us in; a short
                # burst warms the HAM clock without delaying it (the old
                # 16-MM block pushed the first QK to 14us+).
                for w in range(4):
                    wt = psum_s.tile([128, 512], F32, tag="st", name="warm_t")
                    nc.tensor.matmul(wt[:], warm_w[:], warm_r[:],
                                     start=True, stop=True)

                def do_av(pairs):
                    """AV matmuls + normalization fixups for up to two
                    finished pairs, BATCHED into one psum tile with one
                    shared reciprocal: the av/pe4/pt psum tag rotates only
                    2 banks, and every allocation costs a PE<->DVE
                    semaphore round trip -- fewer, fatter allocations keep
                    that ring off the critical path.

                    Emitted one group late so the PE never waits on the
                    exp of the current group (software pipelining)."""
                    n = len(pairs)
                    av = psum_m.tile([NS, 2, C + 1], F32, tag="m", name="av")
                    for i, (h, b, expS, vp_p, pe4_sb_p) in enumerate(pairs):
                        for j in range(J):
                            nc.tensor.matmul(
                                av[:, i, :], expS[:, j, :], vp_p[:, j, b, :],
                                start=(j == 0), stop=(j == J - 1))
                    recip = pool_r.tile([NS, 2, 1], F32, tag="recip",
                                        name="recip")
                    nc.vector.reciprocal(recip[:, 0:n, :],
                                         av[:, 0:n, C:C + 1])
                    for i, (h, b, expS, vp_p, pe4_sb_p) in enumerate(pairs):
                        # x = ctx_exp/den + ctx_pe
                        nc.vector.scalar_tensor_tensor(
                            out=x_nat[b][:, h, :],
                            in0=av[:, i, 0:C],
                            scalar=recip[:, i, 0:1],
                            in1=pe4_sb_p[:, b, 0:C],
                            op0=mybir.AluOpType.mult,
                            op1=mybir.AluOpType.add)


                def qk_mms(items):
                    """Emit j-interleaved QK matmuls for the given
                    (st, qk_t, lo) operands: adjacent matmuls hit
                    disjoint PE row groups (rows 0:64 / 64:128) and run
                    concurrently on the 16-subarray PE."""
                    for j in range(J):
                        for st_t, qk_s, lo in items:
                            nc.tensor.matmul(
                                st_t[:, j, :],
                                qk_s[lo:lo + C, j * 128:(j + 1) * 128],
                                qk_s[lo:lo + C, N:],
                                start=True, stop=True,
                                tile_position=(lo, 0))

                def do_exp(st_t, h, b, vp_p, pe4_p):
                    e = pool_e.tile([128, J, NS], cdt, tag="expS", name="e")
                    nc.scalar.activation(
                        e[:], st_t[:],
                        mybir.ActivationFunctionType.Exp, scale=SCALE)
                    return (h, b, e, vp_p, pe4_p)

                # Software-pipelined B tile: iteration g emits matmuls for
                # (stB of group g-1, stA of group g).  Both tiles' ring
                # buffers were freed >=3 exp-slots earlier, so the QK
                # stream NEVER waits on the exp pacer -- exp(B_{g-1}) can
                # start the instant exp(A_{g-1}) finishes.
                bq = None           # pending B work: (stB, qk_t, h, b, vp, pe4)
                exp_prev = []       # pairs exp'd last iteration -> AV now
                exp_cur = []
                for h in range(H):
                    pe4_sb = pool_pe.tile([NS, B, C + 1], F32, tag="pe4sb",
                                          name="pe4_sb")

                    for bp in range(2):
                        # k^T|q^T for TWO batches stacked on the partition
                        # axis: one full-128-partition DMA per two pairs.
                        if (h, bp) in pre_qk:
                            qk_t = pre_qk[(h, bp)]
                        else:
                            qk_t = pool_k.tile([2 * C, N + NS], qdt,
                                               tag="kT")
                            nc.sync.dma_start(qk_t[:], qk_d[h, bp])
                        if bp == 0:
                            if h == 2:
                                pv_t = pre_pv
                            elif h < 2:
                                pv_t = pre_pv01[h]
                            else:
                                pv_t = pool_v.tile([128, J, PVW], cdt,
                                                   tag="vp", name="pv_t")
                                nc.gpsimd.dma_start(pv_t[:], pv_d[h])
                            peT_t = pv_t[:, :, 0:NS]
                            vp_t = pv_t[:, :, NS:].rearrange(
                                "p j (b c) -> p j b c", b=B)

                        stA = psum_s.tile([128, J, NS], F32, tag="st",
                                          name="stA")
                        stB = psum_s.tile([128, J, NS], F32, tag="st",
                                          name="stB")
                        mms = [(stA, qk_t, 0)]
                        if bq is not None:
                            mms.insert(0, (bq[0], bq[1], C))
                        qk_mms(mms)

                        exp_cur = []
                        if bq is not None:
                            exp_cur.append(do_exp(bq[0], bq[2], bq[3],
                                                  bq[4], bq[5]))
                        exp_cur.append(do_exp(stA, h, 2 * bp, vp_t, pe4_sb))

                        if exp_prev:
                            do_av(exp_prev)
                        exp_prev = exp_cur
                        if h < 4:
                            # ramp-phase filler: keep the PE activity window
                            # full while the pipeline is still shallow
                            for _ in range(8 if h < 2 else 2):
                                wt = psum_s.tile([128, 512], F32, tag="st",
                                                 name="warm_t")
                                nc.tensor.matmul(wt[:], warm_w[:], warm_r[:],
                                                 start=True, stop=True)
                        bq = (stB, qk_t, h, 2 * bp + 1, vp_t, pe4_sb)

                        if bp == 0:
                            # pe @ v for all 4 batches of this head,
                            # emitted after ready PE work so a late vp/peT
                            # DMA can't stall the in-order PE stream.
                            pe4 = psum_m.tile([NS, B, C + 1], F32,
                                              tag="m", name="pe4")
                            for j in range(J):
                                nc.tensor.matmul(
                                    pe4[:], peT_t[:, j, :], vp_t[:, j, :, :],
                                    start=(j == 0), stop=(j == J - 1))
                            # stage in SBUF: DVE may read only one PSUM
                            # input (DMA and GPSIMD cannot read PSUM at all)
                            nc.vector.tensor_copy(pe4_sb[:], pe4[:])
                        elif 2 <= h < 10:
                            # stream two MLP weight chunks per head on the
                            # gpsimd queue -- never on the ACT/sync
                            # sequencers, whose pacing is latency-critical.
                            for wc in range(2):
                                ci = (h - 2) * 2 + wc
                                if ci < D // 128:
                                    nc.gpsimd.dma_start(w1_s[:, ci, :],
                                                        w1_r[:, ci, :])
                                else:
                                    nc.gpsimd.dma_start(
                                        w2_s[:, ci - D // 128, :],
                                        w2_r[:, ci - D // 128, :])
                # drain the pipeline: B of the last group, then final AVs
                qk_mms([(bq[0], bq[1], C)])
                last = do_exp(bq[0], bq[2], bq[3], bq[4], bq[5])
                do_av(exp_prev)
                do_av([last])

            # ---------------- MLP ----------------
            with ExitStack() as mlp_ctx:
                psum_h1 = mlp_ctx.enter_context(
                    tc.tile_pool(name="ph1", bufs=3, space="PSUM"))
                psum_y = mlp_ctx.enter_context(
                    tc.tile_pool(name="py", bufs=4, space="PSUM"))

                # x^T via 32 PE transposes, moved OUT of the attention
                # steady state: the av/pe4 psum tag there rotates only 2
                # banks and every pt allocation cost a PE<->DVE semaphore
                # round trip that stalled the exp pacer.
                for t in range(D // 128):
                    for b in range(B):
                        pt = psum_y.tile([128, NS], cdt, tag="y", name="pt")
                        nc.tensor.transpose(
                            pt[:], x_nat[b][:, 2 * t:2 * t + 2, :], ident[:])
                        nc.vector.tensor_copy(xT[:, t, b, :], pt[:])

                # b2 broadcast tile: [128, D] = ones_col @ b2 (two K=1
                # matmuls); fc2 adds it during the DVE psum eviction.
                for half in range(2):
                    bb = psum_y.tile([128, 512], F32, tag="y", name="bb")
                    nc.tensor.matmul(
                        bb[:], ones1[:1, :],
                        b2_s[:1, half * 512:(half + 1) * 512],
                        start=True, stop=True)
                    nc.vector.tensor_copy(
                        b2b[:, half * 512:(half + 1) * 512], bb[:])

                # fc1: hdn^T[do, rows] = sum_i w1[i]^T.T @ xT[i]
                # Silu(h1 + b1) in ONE fused ACT eviction (bias is
                # per-partition in this layout) -- the old sigmoid+DVE
                # chain made fc1 evacuation-bound.
                for o in range(D // 128):
                    h1 = psum_h1.tile([128, B, NS], F32, tag="h1")
                    for i in range(D // 128):
                        nc.tensor.matmul(
                            h1[:], w1_s[:, i, o * 128:(o + 1) * 128],
                            xT[:, i, :, :],
                            start=(i == 0), stop=(i == D // 128 - 1))
                    nc.scalar.activation(
                        hdnT[:, o, :, :], h1[:],
                        mybir.ActivationFunctionType.Silu,
                        bias=b1_s[:, o:o + 1])

                # fc2: y[rows, do] = sum_i hdnT[i].T @ w2[i]; b2 is added
                # by the DVE during psum eviction (b2b broadcast tile).
                pool_o = mlp_ctx.enter_context(tc.tile_pool(name="o", bufs=4))
                for t in range(B):
                    for nn in range(2):
                        y = psum_y.tile([128, 512], F32, tag="y")
                        for i in range(D // 128):
                            nc.tensor.matmul(
                                y[:], hdnT[:, i, t, :],
                                w2_s[:, i, nn * 512:(nn + 1) * 512],
                                start=(i == 0), stop=(i == D // 128 - 1))
                        y_sb = pool_o.tile([128, 512], F32, tag="ysb")
                        nc.vector.tensor_tensor(
                            out=y_sb[:], in0=y[:],
                            in1=b2b[:, nn * 512:(nn + 1) * 512],
                            op=mybir.AluOpType.add)
                        # alternate output DMAs across two queues so the
                        # final transfers drain in parallel
                        eng = (nc.scalar, nc.sync,
                               nc.gpsimd)[(t * 2 + nn) % 3]
                        eng.dma_start(
                            out_d[t, :, nn * 512:(nn + 1) * 512], y_sb[:])

    nc.compile()
    return nc


_PROG = None


def _get_prog():
    global _PROG
    if _PROG is None:
        _PROG = build_program()
    return _PROG


def _np_dt(cdt):
    if cdt == mybir.dt.bfloat16:
        import ml_dtypes
        return ml_dtypes.bfloat16
    return np.float32


def make_in_maps(q, k, v, pe, w1, b1, w2, b2, cdt=CDT):
    import ml_dtypes
    ndt = _np_dt(cdt)
    qdt8 = ml_dtypes.float8_e4m3
    # [b,h,n,c] -> [h, b//2, (b%2)*C+c, n]
    qT = np.transpose(q, (1, 0, 3, 2)).reshape(H, B // 2, 2 * C, N)
    qT = qT.astype(qdt8)
    kT = np.transpose(k, (1, 0, 3, 2)).reshape(H, B // 2, 2 * C, N)
    kT = kT.astype(qdt8)
    vp = np.concatenate([v, np.ones((B, H, N, 1), v.dtype)], axis=-1)
    vp = np.transpose(vp, (1, 2, 0, 3)).reshape(H, N, B * (C + 1)).astype(ndt)
    peT = np.transpose(pe[0], (0, 2, 1)).astype(ndt)
    # w1/w2 host-swizzled to [p, i, o] with din = i*128+p; b1 to [p, o128]
    w1c = np.ascontiguousarray(
        w1.reshape(D // 128, 128, D).transpose(1, 0, 2)).astype(ndt)
    w2c = np.ascontiguousarray(
        w2.reshape(D // 128, 128, D).transpose(1, 0, 2)).astype(ndt)
    b1f = np.ascontiguousarray(
        b1.reshape(D // 128, 128).T).astype(np.float32)
    b2c = np.ascontiguousarray(b2.reshape(1, D)).astype(ndt)
    idm = np.eye(128, dtype=np.float32).astype(ndt)

    in_maps = []
    for r in range(NCORES):
        sl = slice(r * NS, (r + 1) * NS)
        # kT is full N (not sharded); qT carries this core's q rows
        qk = np.ascontiguousarray(
            np.concatenate([kT, qT[:, :, :, sl]], axis=-1))
        pv = np.concatenate([peT[:, :, sl], vp], axis=-1)
        # swizzle n -> (j, p): [H, N, PVW] -> [H, 128, J, PVW] so the
        # device DMA is contiguous per partition (no DIRECT2D gather).
        pv = np.ascontiguousarray(
            pv.reshape(H, J, 128, PVW).transpose(0, 2, 1, 3))
        in_maps.append({
            "qk": qk,
            "pv": pv,
            "idm": idm,
            "w1s": w1c,
            "b1s": b1f,
            "w2s": w2c,
            "b2s": b2c,
        })
    return in_maps


def assemble(results):
    out = np.empty((B, N, D), np.float32)
    for r in range(NCORES):
        out[:, r * NS:(r + 1) * NS, :] = results[r]["out"]
    return out


def kernel(q, k, v, pe, w1, b1, w2, b2):
    nc = _get_prog()
    in_maps = make_in_maps(q, k, v, pe, w1, b1, w2, b2)
    res = run_bass_kernel_spmd(nc, in_maps, core_ids=list(range(NCORES)))
    return assemble(res.results)
